# revision 5
# baseline (speedup 1.0000x reference)
"""Multi-head causal attention Bass/Tile kernel for TRN2 — v3.

Per-core program (SPMD across 8 cores): each core handles one batch b and
half the heads (HPC=8). Host pre-transposes/slices and casts to bf16.

v3 structure (vs v2): head-PAIR attention restored so the two K=64 QK
matmuls row-tile into the PE array halves (partition bases 0/64) and
stream concurrently; QK/exp run at full 1024 width (bf16 moving operand
max); PSUM = sc_e+sc_o (4 banks) + per-pair ctx split into lo/hi 512-col
halves (4 banks). The ctx lo halves complete early (last contributing key
tile is at the 512-col boundary), are evacuated mid-pair, and their banks
host interleaved projection / output-projection groups. Phases pipeline:
proj(block0) -> attn(qb0) || proj(block1) -> attn(qb1) || outproj(qb0)
-> outproj(qb1). ACT does exp only; DVE does masks/evacs/bias; Pool does
the normalize multiplies; denominator recip chains emit deferred.
"""

from contextlib import ExitStack

import numpy as np

import concourse.bass as bass
import concourse.mybir as mybir
import concourse.tile as tile

F32 = mybir.dt.float32
F32R = mybir.dt.float32r
BF16 = mybir.dt.bfloat16
AF = mybir.ActivationFunctionType


def split_multiwaits(nc):
    """This walrus build accepts at most one sync-wait per instruction;
    hoist extra waits onto NOPs placed just before the instruction."""
    n_split = 0
    for fn in nc.m.functions:
        for blk in fn.blocks:
            insts = list(blk.instructions)
            out = []
            for inst in insts:
                si = inst.sync_info
                if si is not None and si.on_wait is not None and len(si.on_wait) > 1:
                    waits = list(si.on_wait)
                    for j, w in enumerate(waits[:-1]):
                        nop = mybir.InstNoOp(name=f"{inst.name}-sw{j}", ins=[], outs=[])
                        nop.engine = inst.engine
                        nop.sync_info = mybir.SyncInfo(on_wait=[w], on_update=[])
                        out.append(nop)
                    inst.sync_info = mybir.SyncInfo(
                        on_wait=[waits[-1]], on_update=list(si.on_update or [])
                    )
                    n_split += 1
                out.append(inst)
            if len(out) != len(insts):
                blk.instructions.clear()
                blk.instructions.extend(out)
    return n_split


def build(S=2048, D=1024, HPC=8, DK=64, DO=1024, QB=1024, QC=512, scale=0.125,
          split=True):
    assert S == 2 * QB and QB == 2 * QC
    n_dt = D // 128
    n_st = S // 128
    n_qb = S // QB
    n_pairs = HPC // 2
    HD = HPC * DK
    n_ht = HD // 128
    VA = DK + 1
    FQ = QB // 128

    nc = bass.Bass("TRN2", target_bir_lowering=False, debug=False)

    xqT = nc.dram_tensor("xqT", [D, S], BF16, kind="ExternalInput").ap()
    xkT = nc.dram_tensor("xkT", [D, S], BF16, kind="ExternalInput").ap()
    xvT = nc.dram_tensor("xvT", [D, S], BF16, kind="ExternalInput").ap()
    wq = nc.dram_tensor("wq", [D, HD], BF16, kind="ExternalInput").ap()
    wk = nc.dram_tensor("wk", [D, HD], BF16, kind="ExternalInput").ap()
    wv = nc.dram_tensor("wv", [D, HD], BF16, kind="ExternalInput").ap()
    bqp = nc.dram_tensor("bqp", [2 * DK, n_pairs], F32, kind="ExternalInput").ap()
    bkp = nc.dram_tensor("bkp", [2 * DK, n_pairs], F32, kind="ExternalInput").ap()
    wo = nc.dram_tensor("wo", [HD, DO], BF16, kind="ExternalInput").ap()
    masks_in = nc.dram_tensor("masks_in", [(QC // 128) * 128, QC], BF16,
                              kind="ExternalInput").ap()
    vones = nc.dram_tensor("vones", [128, HPC, 1], BF16, kind="ExternalInput").ap()
    out = nc.dram_tensor("out", [S, DO], BF16, kind="ExternalOutput").ap()

    with tile.TileContext(nc) as tc:
        ctx = ExitStack()
        qk_pool = ctx.enter_context(tc.tile_pool(name="qk", bufs=1))
        va_pool = ctx.enter_context(tc.tile_pool(name="va", bufs=1))
        cs_pool = ctx.enter_context(tc.tile_pool(name="cs", bufs=1))
        small_pool = ctx.enter_context(tc.tile_pool(name="small", bufs=1))
        wpool = ctx.enter_context(tc.tile_pool(name="w", bufs=3))
        xpool = ctx.enter_context(tc.tile_pool(name="xin", bufs=3))
        expool = ctx.enter_context(tc.tile_pool(name="ex", bufs=1))
        rcpool = ctx.enter_context(tc.tile_pool(name="rc", bufs=3))
        csbpool = ctx.enter_context(tc.tile_pool(name="csb", bufs=3))
        dscrpool = ctx.enter_context(tc.tile_pool(name="dscr", bufs=8, space="DRAM"))
        wopool = ctx.enter_context(tc.tile_pool(name="wop", bufs=1))
        oev = ctx.enter_context(tc.tile_pool(name="oev", bufs=4))
        patt = ctx.enter_context(tc.tile_pool(name="patt", bufs=1, space="PSUM"))

        qt_sb = [qk_pool.tile([2 * DK, S], BF16, name=f"qt{p}", tag=f"qt{p}")
                 for p in range(n_pairs)]
        kt_sb = [qk_pool.tile([2 * DK, S], BF16, name=f"kt{p}", tag=f"kt{p}")
                 for p in range(n_pairs)]
        v_aug = [va_pool.tile([128, HPC * VA], BF16, name=f"va{t}", tag=f"va{t}")
                 for t in range(n_st)]
        ctx_stack = [cs_pool.tile([128, S], BF16, name=f"cs{t}", tag=f"cs{t}")
                     for t in range(n_ht)]

        bq_sb = small_pool.tile([2 * DK, n_pairs], F32, tag="bq")
        bk_sb = small_pool.tile([2 * DK, n_pairs], F32, tag="bk")
        masks = [small_pool.tile([128, QC], BF16, name=f"mask{r}", tag=f"mask{r}")
                 for r in range(QC // 128)]

        vones_sb = small_pool.tile([128, HPC], BF16, tag="vones")

        def small_loads():
            nc.sync.dma_start(bq_sb[:], bqp[:])
            nc.sync.dma_start(bk_sb[:], bkp[:])
            nc.sync.dma_start(vones_sb[:], vones[:, :, 0])
            for r in range(QC // 128):
                nc.sync.dma_start(masks[r][:], masks_in[r * 128:(r + 1) * 128, :])

        # weight tiles load just-in-time: each projection's 8 tiles are
        # DMA'd by that projection's first loader closure (see
        # proj_block_groups); wo loads defer to attention start.
        w_t = {}
        w_loaded = set()

        def w_load(nm):
            if nm in w_loaded:
                return
            w_loaded.add(nm)
            w = {"q": wq, "k": wk, "v": wv}[nm]
            wt = wpool.tile([128, n_dt * HD], BF16, name=f"w{nm}", tag="w")
            nc.sync.dma_start(wt[:].rearrange("p (d h) -> p d h", d=n_dt),
                              w[:].rearrange("(d p) h -> p d h", p=128))
            for d in range(n_dt):
                w_t[(nm, d)] = wt[:, d * HD:(d + 1) * HD]

        wo_t = []

        def wo_load():
            for t in range(n_ht):
                wt = wopool.tile([128, DO], BF16, name=f"wo{t}", tag=f"wo{t}")
                nc.sync.dma_start(wt[:], wo[t * 128:(t + 1) * 128, :])
                wo_t.append(wt)

        # rotating prob buffers (memset once: the mask multiply reads the
        # stale region below the causal start and relies on finite values)
        ex_tiles = {}
        for eo in range(2):
            for i in range(3):
                exb = expool.tile([128, QB], BF16, name=f"exb{eo}_{i}",
                                  tag=f"ex{eo}_{i}")
                nc.any.memset(exb[:], 0)

        xT_map = {"q": xqT, "k": xkT, "v": xvT}

        def x_load(nm, blk, half):
            w_load(nm)
            c0 = blk * QB + half * QC
            xt = xpool.tile([128, n_dt * QC], BF16, name=f"x{nm}{blk}{half}",
                            tag="x")
            nc.sync.dma_start(
                xt[:].rearrange("p (d c) -> p d c", d=n_dt),
                xT_map[nm][:, c0:c0 + QC].rearrange("(d p) c -> p d c", p=128))
            return [xt[:, d * QC:(d + 1) * QC] for d in range(n_dt)]

        import itertools
        _bnk_ctr = itertools.count()

        def bank_tile(slot):
            # the two "flex" PSUM banks: ctx lo halves / proj / outproj
            return patt.tile([128, QC], F32, name=f"bnk_{next(_bnk_ctr)}",
                             tag=f"bnk{slot % 2}")

        def proj_qk_group(nm, dst, b_sb, blk, half, p, x_tiles):
            ps = bank_tile(p)
            for d in range(n_dt):
                nc.tensor.matmul(ps[:], w_t[(nm, d)][:, p * 128:(p + 1) * 128],
                                 x_tiles[d][:], start=(d == 0), stop=(d == n_dt - 1))
            c0 = blk * QB + half * QC
            nc.vector.tensor_scalar_add(dst[p][:, c0:c0 + QC], ps[:],
                                        b_sb[:, p:p + 1])

        def proj_v_group(blk, half, stl, x_tiles):
            ps = bank_tile(stl)
            for d in range(n_dt):
                nc.tensor.matmul(ps[:, 0:HD],
                                 x_tiles[d][:, stl * 128:(stl + 1) * 128],
                                 w_t[("v", d)][:], start=(d == 0),
                                 stop=(d == n_dt - 1))
            st = blk * (QB // 128) + half * (QC // 128) + stl
            va3 = v_aug[st][:].rearrange("p (h c) -> p h c", c=VA)
            nc.vector.tensor_copy(va3[:, :, DK:VA],
                                  vones_sb[:].rearrange("p h -> p h ()"))
            nc.vector.tensor_copy(va3[:, :, 0:DK],
                                  ps[:, 0:HD].rearrange("p (h c) -> p h c", c=DK))

        def proj_block_groups(blk):
            """Closures for one 1024-col block of Q/K/V projections.

            x-tile DMA loaders are interleaved ~4 pops ahead of the matmul
            groups that consume them so the in-order PE queue never waits
            on a just-issued DMA."""
            units = []   # (kind, payload) in consumption order
            for (nm, dst, b_sb) in (("k", kt_sb, bk_sb), ("v", None, None),
                                    ("q", qt_sb, bq_sb)):
                for half in range(2):
                    box = {}
                    ld = (lambda nm=nm, blk=blk, half=half, box=box:
                          box.setdefault("x", x_load(nm, blk, half)))
                    gs = []
                    if nm == "v":
                        for stl in range(QC // 128):
                            gs.append(lambda stl=stl, blk=blk, half=half,
                                      box=box: proj_v_group(blk, half, stl,
                                                            box["x"]))
                    else:
                        for p in range(n_pairs):
                            gs.append(lambda p=p, nm=nm, dst=dst, b_sb=b_sb,
                                      blk=blk, half=half, box=box:
                                      proj_qk_group(nm, dst, b_sb, blk, half,
                                                    p, box["x"]))
                    units.append((ld, gs))
            # interleave: issue loader i+1 before the groups of loader i
            closures = [units[0][0], units[1][0]]
            for i, (_, gs) in enumerate(units):
                if i + 2 < len(units):
                    closures.append(units[i + 2][0])
                closures.extend(gs)
            return closures

        def outproj_group(qb, g, four_way=False):
            stl, nck = g // 2, g % 2
            st = qb * FQ + stl
            if four_way and g % 4 >= 2:
                ps = patt.tile([128, QC], F32, name=f"opx_{qb}_{g}",
                               tag=("ctxeh" if g % 2 == 0 else "ctxoh"))
            else:
                ps = bank_tile(g)
            for t in range(n_ht):
                nc.tensor.matmul(ps[:], ctx_stack[t][:, st * 128:(st + 1) * 128],
                                 wo_t[t][:, nck * QC:(nck + 1) * QC],
                                 start=(t == 0), stop=(t == n_ht - 1))
            ev = oev.tile([128, QC], BF16, tag="ev")
            nc.vector.tensor_copy(ev[:], ps[:])
            nc.sync.dma_start(out[st * 128:(st + 1) * 128,
                                  nck * QC:(nck + 1) * QC], ev[:])

        def emit_denom_half(h, qb, cs, half):
            p, eo = h // 2, h % 2
            FH = QC // 128
            c0, c1 = half * QC, (half + 1) * QC
            ds1 = dscrpool.tile([1, QC], F32, name=f"dsh1_{h}_{qb}_{half}",
                                tag="ds1")
            nc.sync.dma_start(ds1[:], cs[DK:VA, c0:c1])
            dnp = rcpool.tile([128, FH], F32, name=f"dnph{h}_{qb}_{half}",
                              tag="dnp")
            nc.sync.dma_start(dnp[:], ds1[0, :].rearrange("(p f) -> p f", f=FH))
            rcp = rcpool.tile([128, FH], F32R, name=f"rcph{h}_{qb}_{half}",
                              tag="rcp")
            with nc.allow_low_precision(reason="denom recip"):
                nc.vector.reciprocal(rcp[:], dnp[:])
            ds2 = dscrpool.tile([1, QC], F32R, name=f"dsh2_{h}_{qb}_{half}",
                                tag="ds2")
            nc.sync.dma_start(ds2[0, :].rearrange("(p f) -> p f", f=FH), rcp[:])
            bc = rcpool.tile([DK, QC], F32R, name=f"bch{h}_{qb}_{half}", tag="bc")
            nc.sync.dma_start(bc[:], ds2[:].broadcast_to([DK, QC]))
            nc.gpsimd.tensor_mul(
                ctx_stack[p][eo * DK:(eo + 1) * DK,
                             qb * QB + c0:qb * QB + c1],
                cs[0:DK, c0:c1], bc[:])

        deferred = []

        def flush_deferred(n):
            for _ in range(min(n, len(deferred))):
                deferred.pop(0)()

        def make_denom_chain(h, qb, cs):
            p, eo = h // 2, h % 2

            def emit():
                ds1 = dscrpool.tile([1, QB], F32, name=f"ds1_{h}_{qb}", tag="ds1")
                nc.sync.dma_start(ds1[:], cs[DK:VA, :])
                dnp = rcpool.tile([128, FQ], F32, name=f"dnp{h}_{qb}", tag="dnp")
                nc.sync.dma_start(dnp[:], ds1[0, :].rearrange("(p f) -> p f", f=FQ))
                rcp = rcpool.tile([128, FQ], F32R, name=f"rcp{h}_{qb}", tag="rcp")
                with nc.allow_low_precision(reason="denom recip"):
                    nc.vector.reciprocal(rcp[:], dnp[:])
                ds2 = dscrpool.tile([1, QB], F32R, name=f"ds2_{h}_{qb}", tag="ds2")
                nc.sync.dma_start(ds2[0, :].rearrange("(p f) -> p f", f=FQ), rcp[:])
                bc = rcpool.tile([DK, QB], F32R, name=f"bc{h}_{qb}", tag="bc")
                nc.sync.dma_start(bc[:], ds2[:].broadcast_to([DK, QB]))
                nc.gpsimd.tensor_mul(
                    ctx_stack[p][eo * DK:(eo + 1) * DK, qb * QB:(qb + 1) * QB],
                    cs[0:DK, :], bc[:])
            return emit

        svc = []  # service queue: proj/outproj group closures

        def pop_svc(n):
            for _ in range(min(n, len(svc))):
                svc.pop(0)()

        # ---- phase 0: block0 projections (K, V, Q for seq cols 0:1024) ----
        blk0 = proj_block_groups(0)
        blk0[0]()          # first x/w loader starts streaming immediately
        small_loads()      # constants ride behind the first big DMA
        for g in blk0[1:]:
            g()
        wo_load()
        svc.extend(proj_block_groups(1))

        # ---- attention ----
        for qb in range(n_qb):
            ktm = ((qb + 1) * QB) // 128 - 1
            lk_lo = (qb * QB + QC) // 128 - 1   # last kt touching the lo half
            for p in range(n_pairs):
                qt_e = qt_sb[p][0:DK, :]
                qt_o = qt_sb[p][DK:2 * DK, :]
                kt_e = kt_sb[p][0:DK, :]
                kt_o = kt_sb[p][DK:2 * DK, :]
                he, ho = 2 * p, 2 * p + 1

                ctx_lo = [bank_tile(0), bank_tile(1)]     # [e, o] cols 0:QC
                ctx_hi = [patt.tile([VA, QC], F32, name=f"cxh0_{qb}_{p}",
                                    tag="ctxeh"),
                          patt.tile([VA, QC], F32, name=f"cxh1_{qb}_{p}",
                                    tag="ctxoh")]
                cs_t = [csbpool.tile([VA, QB], F32, name=f"cse{qb}_{he}",
                                     tag="cse"),
                        csbpool.tile([VA, QB], F32, name=f"cso{qb}_{ho}",
                                     tag="cse")]

                def emit_av(kt, _ctx_lo=ctx_lo, _ctx_hi=ctx_hi, _qb=qb, _he=he,
                            _ho=ho, _ktm=ktm):
                    rel_lo = max(0, kt * 128 - _qb * QB)
                    qc0 = (rel_lo // QC) * QC
                    for c in range(qc0, QB, QC):
                        half = c // QC
                        last_kt = min(_ktm, (_qb * QB + c + QC) // 128 - 1)
                        for eo, hh in ((0, _he), (1, _ho)):
                            dst = (_ctx_lo[eo][0:VA, :] if half == 0
                                   else _ctx_hi[eo][:])
                            nc.tensor.matmul(
                                dst, v_aug[kt][:, hh * VA:(hh + 1) * VA],
                                ex_tiles[(eo, kt % 3)][:, c:c + QC],
                                start=(kt == 0), stop=(kt == last_kt))

                for kt in range(ktm + 1):
                    rel_lo = max(0, kt * 128 - qb * QB)
                    qc0 = (rel_lo // QC) * QC
                    sc_e = patt.tile([128, QB], F32, name=f"sce{kt}", tag="sce")
                    sc_o = patt.tile([128, QB], F32, name=f"sco{kt}", tag="sco")
                    for c in range(qc0, QB, QC):
                        nc.tensor.matmul(sc_e[:, c:c + QC],
                                         kt_e[:, kt * 128:(kt + 1) * 128],
                                         qt_e[:, qb * QB + c:qb * QB + c + QC],
                                         start=True, stop=True)
                        nc.tensor.matmul(sc_o[:, c:c + QC],
                                         kt_o[:, kt * 128:(kt + 1) * 128],
                                         qt_o[:, qb * QB + c:qb * QB + c + QC],
                                         start=True, stop=True)
                    ex_e = expool.tile([128, QB], BF16, name=f"exe{kt}",
                                       tag=f"ex0_{kt % 3}")
                    ex_o = expool.tile([128, QB], BF16, name=f"exo{kt}",
                                       tag=f"ex1_{kt % 3}")
                    ex_tiles[(0, kt % 3)] = ex_e
                    ex_tiles[(1, kt % 3)] = ex_o
                    nc.scalar.activation(ex_e[:, rel_lo:QB], sc_e[:, rel_lo:QB],
                                         AF.Exp, scale=scale)
                    nc.scalar.activation(ex_o[:, rel_lo:QB], sc_o[:, rel_lo:QB],
                                         AF.Exp, scale=scale)
                    if kt * 128 >= qb * QB:
                        r = (rel_lo - qc0) // 128
                        nc.vector.tensor_mul(ex_e[:, qc0:qc0 + QC],
                                             ex_e[:, qc0:qc0 + QC], masks[r][:])
                        nc.vector.tensor_mul(ex_o[:, qc0:qc0 + QC],
                                             ex_o[:, qc0:qc0 + QC], masks[r][:])
                    if kt >= 1:
                        emit_av(kt - 1)
                    if kt == lk_lo + 2:
                        # lo halves complete (AV(lk_lo) emitted last iter):
                        # evacuate, freeing the two flex banks
                        for eo in range(2):
                            nc.vector.tensor_copy(cs_t[eo][:, 0:QC],
                                                  ctx_lo[eo][0:VA, :])
                    if kt == 2:
                        flush_deferred(2)
                    if kt > lk_lo + 2:
                        pop_svc(2 if qb else 3)
                emit_av(ktm)
                last = (qb == n_qb - 1 and p == n_pairs - 1)
                if last:
                    for eo, hh in ((0, he), (1, ho)):
                        emit_denom_half(hh, qb, cs_t[eo], 0)
                for eo, hh in ((0, he), (1, ho)):
                    nc.vector.tensor_copy(cs_t[eo][:, QC:QB], ctx_hi[eo][:])
                    if last:
                        deferred.append(
                            (lambda hh=hh, qb=qb, eo=eo:
                             emit_denom_half(hh, qb, cs_t[eo], 1)))
                    else:
                        deferred.append(make_denom_chain(hh, qb, cs_t[eo]))
            if qb == 0:
                pop_svc(len(svc))   # any leftover block1 proj groups
                svc.extend([(lambda g=g: outproj_group(0, g)) for g in range(2 * FQ)])

        flush_deferred(len(deferred))
        pop_svc(len(svc))
        drain_ps = {}
        for wave in range(0, 2 * FQ, 4):
            gs = range(wave, min(wave + 4, 2 * FQ))
            for g in gs:
                stl, nck = g // 2, g % 2
                st = FQ + stl
                if g % 4 >= 2:
                    ps = patt.tile([128, QC], F32, name=f"opw_{g}",
                                   tag=("ctxeh" if g % 2 == 0 else "ctxoh"))
                else:
                    ps = bank_tile(g)
                drain_ps[g] = ps
                for t in range(n_ht - 1):
                    nc.tensor.matmul(ps[:],
                                     ctx_stack[t][:, st * 128:(st + 1) * 128],
                                     wo_t[t][:, nck * QC:(nck + 1) * QC],
                                     start=(t == 0), stop=False)
            for g in gs:
                stl, nck = g // 2, g % 2
                st = FQ + stl
                t = n_ht - 1
                nc.tensor.matmul(drain_ps[g][:],
                                 ctx_stack[t][:, st * 128:(st + 1) * 128],
                                 wo_t[t][:, nck * QC:(nck + 1) * QC],
                                 start=False, stop=True)
                ev = oev.tile([128, QC], BF16, name=f"evw_{g}", tag="ev")
                nc.scalar.copy(ev[:], drain_ps[g][:])
                nc.sync.dma_start(out[st * 128:(st + 1) * 128,
                                      nck * QC:(nck + 1) * QC], ev[:])
        ctx.close()

    if split:
        split_multiwaits(nc)
    return nc


def core_inputs(queries, keys, values, Wq, bq, Wk, bk, Wv, bv, Wo, core, n_cores=8,
                HPC=None):
    import ml_dtypes
    B = queries.shape[0]
    H = Wq.shape[0]
    groups = n_cores // B
    b, hg = core // groups, core % groups
    if HPC is None:
        HPC = H // groups
    h0 = hg * HPC
    DK = Wq.shape[2]
    bf16 = ml_dtypes.bfloat16

    def wsel(W):
        return np.ascontiguousarray(
            W[h0:h0 + HPC].transpose(1, 0, 2).reshape(W.shape[1], HPC * DK)
        ).astype(bf16)

    def bpairs(bias):
        bsel = bias[h0:h0 + HPC].reshape(HPC // 2, 2 * DK)
        return np.ascontiguousarray(bsel.T)

    QC = 512
    n_r = QC // 128
    masks_in = np.zeros((n_r * 128, QC), np.float32)
    for r in range(n_r):
        x = np.arange(128)[:, None]
        y = np.arange(QC)[None, :]
        masks_in[r * 128:(r + 1) * 128] = (y - x - 128 * r >= 0).astype(np.float32)
    return {
        "masks_in": masks_in.astype(bf16),
        "vones": np.ones((128, HPC, 1), bf16),
        "xqT": np.ascontiguousarray(queries[b].T).astype(bf16),
        "xkT": np.ascontiguousarray(keys[b].T).astype(bf16),
        "xvT": np.ascontiguousarray(values[b].T).astype(bf16),
        "wq": wsel(Wq), "wk": wsel(Wk), "wv": wsel(Wv),
        "bqp": bpairs(bq), "bkp": bpairs(bk),
        "wo": np.ascontiguousarray(Wo[h0 * DK:(h0 + HPC) * DK, :]).astype(bf16),
    }


def assemble(results, B, n_cores, bias_total):
    groups = n_cores // B
    outs = []
    for b in range(B):
        acc = results[b * groups]["out"].astype(np.float64)
        for g in range(1, groups):
            acc = acc + results[b * groups + g]["out"]
        outs.append(acc + bias_total)
    return np.stack(outs).astype(np.float32)


_CACHE = {}


def kernel(**inputs):
    from concourse.bass_utils import run_bass_kernel_spmd

    queries = np.asarray(inputs["queries"], np.float32)
    keys = np.asarray(inputs["keys"], np.float32)
    values = np.asarray(inputs["values"], np.float32)
    Wq = np.asarray(inputs["Wq"], np.float32)
    bq = np.asarray(inputs["bq"], np.float32)
    Wk = np.asarray(inputs["Wk"], np.float32)
    bk = np.asarray(inputs["bk"], np.float32)
    Wv = np.asarray(inputs["Wv"], np.float32)
    bv = np.asarray(inputs["bv"], np.float32)
    Wo = np.asarray(inputs["Wo"], np.float32)
    bo = np.asarray(inputs["bo"], np.float32)

    B = queries.shape[0]
    n_cores = 8
    if "nc" not in _CACHE:
        _CACHE["nc"] = build()
    nc = _CACHE["nc"]
    in_maps = [core_inputs(queries, keys, values, Wq, bq, Wk, bk, Wv, bv, Wo,
                           core=c, n_cores=n_cores) for c in range(n_cores)]
    res = run_bass_kernel_spmd(nc, in_maps, list(range(n_cores)))
    bias_total = bo + bv.reshape(-1) @ Wo
    return assemble(res.results, B, n_cores, bias_total)
